# revision 9
# baseline (speedup 1.0000x reference)
"""Bass/Trainium2 kernel for the 2-layer GAT + pair-MLP problem.

Self-contained: builds an edge/pair schedule from the actual inputs,
compiles one SPMD Bass program, runs it on 8 NeuronCores, reassembles the
full [P, 2] output.

Sharding: nodes are split into 8 contiguous shards (dst-owner cores);
each core aggregates all edges whose destination lies in its shard, with
the full per-layer feature table replicated in its HBM (fp16). The pair
batch is split 1/8 per core. Inter-layer node features are exchanged with
an on-device AllGather.

Per GAT layer (all on-device, fp16 data / f32 accumulation):
  ftab = h @ [W | W@al_blockdiag | 0pad]     # [N,384] fp16: feat(256)+el(2)+pad
  er_all = h_shard @ (W@ar_blockdiag)        # [128, NG, 2] resident in SBUF
  per edge run (<=RCAP chunks of 128 edges, same 128-dst group, same src half):
    dma_gather ftab rows by src (the only per-edge gather)
    one-hot oh[e,d] and its transpose ohT[d,e] built as single broadcast
      is_equal ops (fp16); er per edge = ohT-matmul against er_all[:,g,:]
    s = exp(leaky_relu(el+er)) in f32, written into row cols 258:260 (fp16)
    msg = feat * s (per-head broadcast, fp16)
    oh matmul accumulates [128dst, 260] into PSUM (f32)
  out[d,h] = relu(acc_h * recip(den_h + 1e-9)); h[d] = mean over heads (fp16)
"""
import numpy as np
BF16 = np.float16  # 2-byte element dtype for tables/messages

# problem constants (shapes the grader feeds)
N = 50000
E = 800000
IN = 128
NH = 128
H = 2
P = 100000
C = 2
NEG = 0.2

NCORES = 8
NS = N // NCORES          # nodes per shard
GRP = 128                 # dst group size (one PSUM accumulation)
HALF = 32768              # int16 gather index range split
RFB = 384                 # ftab row fp16 cols: 256 feat + 2 el + 2 s + pad
NAGG = 260                # aggregated row cols: 256 msg + 2 el(dead) + 2 s
PADDST = 200.0            # one-hot miss marker for padded edges
RCAP = 8                  # max chunks per gather sub-run (SBUF bound)

_CACHE = {}
_TRACE = False      # set by test harnesses to capture an NTFF profile
LAST_PERF = None


def _ng():
    return -(-NS // GRP)


def _ceil_div(a, b):
    return -(-a // b)


def _wrap_idx16(idx):
    """int16 idx j -> [(j%16) (+16k replicas), j//16] as [128, n/16]."""
    idx = np.asarray(idx, np.int16)
    assert idx.size % 16 == 0
    w = idx.reshape(-1, 16).T
    return np.tile(w, (8, 1)).copy()


def _fold_weights(W, al, ar):
    """Wcat [d, RFB] = [W | W@al_blk | 0] (fp16); Wer [d, 2] = W@ar_blk."""
    d = W.shape[0]
    Wcat = np.zeros((d, RFB), np.float32)
    Wer = np.zeros((d, 2), np.float32)
    Wcat[:, : H * NH] = W
    for h in range(H):
        Wcat[:, H * NH + h] = W[:, h * NH : (h + 1) * NH] @ al[h]
        Wer[:, h] = W[:, h * NH : (h + 1) * NH] @ ar[h]
    return Wcat.astype(BF16), Wer.astype(BF16)


def _build_edge_schedule(src, dst):
    """Uniform (group, bucket) chunk schedule shared by all cores."""
    NG = _ng()
    src = np.asarray(src, np.int64)
    dst = np.asarray(dst, np.int64)
    core = dst // NS
    dloc = dst - core * NS
    g = dloc // GRP
    b = (src >= HALF).astype(np.int64)
    key = (core * NG + g) * 2 + b
    counts = np.bincount(key, minlength=NCORES * NG * 2).reshape(NCORES, NG, 2)
    cpg = np.maximum(1, _ceil_div(counts.max(axis=0), 128))  # [NG, 2]
    run_chunk_start = np.concatenate([[0], np.cumsum(cpg.reshape(-1))])
    tot_chunks = int(run_chunk_start[-1])
    tot_edges = tot_chunks * 128

    order = np.argsort(key, kind="stable")
    key_s = key[order]
    seg_start = np.searchsorted(key_s, np.arange(NCORES * NG * 2))
    rank = np.arange(src.size) - seg_start[key_s]
    gb = key_s % (NG * 2)
    pos = run_chunk_start[gb] * 128 + rank
    core_s = key_s // (NG * 2)

    cores = []
    for c in range(NCORES):
        m = core_s == c
        e = order[m]
        p = pos[m]
        esrc16 = np.zeros(tot_edges, np.int16)
        edstf = np.full(tot_edges, PADDST, np.float32)
        esrc16[p] = (src[e] - b[e] * HALF).astype(np.int16)
        edstf[p] = (dloc[e] % GRP).astype(np.float32)
        edstf = edstf.astype(BF16)
        cores.append(dict(
            esrc16=_wrap_idx16(esrc16),
            edstf=edstf.reshape(tot_chunks, 128).T.copy(),
            edstf_flat=edstf.reshape(1, tot_edges).copy(),
        ))
    return cpg, cores


def _build_head_schedule(x1, x2):
    """4 buckets by (x1>=HALF, x2>=HALF); uniform chunk counts across cores."""
    x1 = np.asarray(x1, np.int64)
    x2 = np.asarray(x2, np.int64)
    PC = P // NCORES
    q_all = (x1 >= HALF).astype(np.int64) + 2 * (x2 >= HALF).astype(np.int64)
    pb = np.ones(4, np.int64)
    for c in range(NCORES):
        cnt = np.bincount(q_all[c * PC : (c + 1) * PC], minlength=4)
        pb = np.maximum(pb, _ceil_div(cnt, 128))
    starts = np.concatenate([[0], np.cumsum(pb)]) * 128
    tot = int(starts[-1])
    percore = []
    for c in range(NCORES):
        sl = slice(c * PC, (c + 1) * PC)
        q = q_all[sl]
        x1c, x2c = x1[sl], x2[sl]
        x1_16 = np.zeros(tot, np.int16)
        x2_16 = np.zeros(tot, np.int16)
        posmap = np.zeros(PC, np.int64)
        for qq in range(4):
            m = q == qq
            n = int(m.sum())
            p = starts[qq] + np.arange(n)
            x1_16[p] = (x1c[m] - (qq & 1) * HALF).astype(np.int16)
            x2_16[p] = (x2c[m] - ((qq >> 1) & 1) * HALF).astype(np.int16)
            posmap[np.nonzero(m)[0]] = p
        percore.append(dict(
            x1_16=_wrap_idx16(x1_16),
            x2_16=_wrap_idx16(x2_16),
            posmap=posmap,
        ))
    return pb, percore


def _build_program(cpg, pb):
    import concourse.bacc as bacc
    import concourse.bass as bass
    import concourse.mybir as mybir
    import concourse.tile as tile

    F32 = mybir.dt.float32
    BF = mybir.dt.float16
    I16 = mybir.dt.int16
    AF = mybir.ActivationFunctionType
    OP = mybir.AluOpType

    NG = _ng()
    NKCH = _ceil_div(N, 128)        # node chunks for ftab builds
    NPAD = NKCH * 128
    tot_chunks = int(cpg.sum())
    tot_e16 = tot_chunks * 8        # idx cols (128 edges -> 8 cols of 16)
    tot_edges = tot_chunks * 128
    hb = int(pb.sum())              # head chunks per core
    TOTP = hb * 128
    KC = 8                          # ftab staging chunks per DMA

    nc = bacc.Bacc("TRN2", target_bir_lowering=False, debug=False,
                   num_devices=NCORES)

    # ---- I/O ----
    hT_d = nc.dram_tensor("hT", [128, N], BF, kind="ExternalInput")
    hTs_d = nc.dram_tensor("hTs", [128, NS], BF, kind="ExternalInput")
    wcat_d = [nc.dram_tensor(f"wcat{l}", [128, RFB], BF, kind="ExternalInput")
              for l in range(2)]
    wer_d = [nc.dram_tensor(f"wer{l}", [128, 2], BF, kind="ExternalInput")
             for l in range(2)]
    w1_d = nc.dram_tensor("w1", [3 * NH, NH], BF, kind="ExternalInput")
    b1_d = nc.dram_tensor("b1", [NH, 1], F32, kind="ExternalInput")
    w2_d = nc.dram_tensor("w2", [NH, C], BF, kind="ExternalInput")
    b2_d = nc.dram_tensor("b2", [C, 1], F32, kind="ExternalInput")
    iota_d = nc.dram_tensor("iota", [128, 128], BF, kind="ExternalInput")
    colv_d = nc.dram_tensor("colv", [128, 128], BF, kind="ExternalInput")
    ident_d = nc.dram_tensor("ident", [128, 128], BF, kind="ExternalInput")
    esrc_d = nc.dram_tensor("esrc", [128, tot_e16], I16, kind="ExternalInput")
    edstf_d = nc.dram_tensor("edstf", [128, tot_chunks], BF,
                             kind="ExternalInput")
    edstfl_d = nc.dram_tensor("edstfl", [1, tot_edges], BF,
                              kind="ExternalInput")
    x1_d = nc.dram_tensor("x1i", [128, hb * 8], I16, kind="ExternalInput")
    x2_d = nc.dram_tensor("x2i", [128, hb * 8], I16, kind="ExternalInput")
    out_d = nc.dram_tensor("headout", [C, TOTP], F32, kind="ExternalOutput")

    with tile.TileContext(nc) as tc:
        with (
            tc.tile_pool(name="dram", bufs=1, space="DRAM") as dp,
            tc.tile_pool(name="cst", bufs=1) as cst,
            tc.tile_pool(name="ld", bufs=3) as ld,
            tc.tile_pool(name="st", bufs=2) as st,
            tc.tile_pool(name="ed", bufs=3) as ed,
            tc.tile_pool(name="hg", bufs=2) as hgp,
            tc.tile_pool(name="sc", bufs=4) as sc,
            tc.tile_pool(name="psA", bufs=3, space="PSUM") as psA,
            tc.tile_pool(name="psB", bufs=2, space="PSUM") as psB,
            tc.tile_pool(name="psE", bufs=2, space="PSUM") as psE,
        ):
            # ---- internal DRAM (pool tiles so Tile tracks RAW deps) ----
            ftab = [dp.tile([NPAD, RFB], BF, name=f"ftab{l}") for l in range(2)]
            hshard = [dp.tile([NS, 128], BF, name=f"hshard{l}")
                      for l in range(2)]
            hfull = dp.tile([N, 128], BF, name="hfull", addr_space="Shared")
            h2full = dp.tile([N, 128], BF, name="h2full", addr_space="Shared")

            # ---- constants ----
            iota_t = cst.tile([128, 128], BF)
            nc.sync.dma_start(iota_t[:], iota_d[:])
            colv_t = cst.tile([128, 128], BF)
            nc.sync.dma_start(colv_t[:], colv_d[:])
            ident_t = cst.tile([128, 128], BF)
            nc.sync.dma_start(ident_t[:], ident_d[:])
            wcat_t, wer_t = [], []
            for l in range(2):
                w = cst.tile([128, RFB], BF, name=f"wcat{l}t")
                nc.sync.dma_start(w[:], wcat_d[l][:])
                wcat_t.append(w)
                w = cst.tile([128, 2], BF, name=f"wer{l}t")
                nc.sync.dma_start(w[:], wer_d[l][:])
                wer_t.append(w)
            er_all = [cst.tile([128, NG, 2], BF, name=f"erall{l}")
                      for l in range(2)]
            w1_t = cst.tile([128, 3, NH], BF)
            for j in range(3):
                nc.sync.dma_start(w1_t[:, j, :], w1_d[j * 128 : (j + 1) * 128, :])
            b1_t = cst.tile([NH, 1], F32)
            nc.sync.dma_start(b1_t[:], b1_d[:])
            w2_t = cst.tile([NH, C], BF)
            nc.sync.dma_start(w2_t[:], w2_d[:])
            b2_t = cst.tile([C, 1], F32)
            nc.sync.dma_start(b2_t[:], b2_d[:])
            edstf_t = cst.tile([128, tot_chunks], BF)
            nc.sync.dma_start(edstf_t[:], edstf_d[:])
            esrc_t = cst.tile([128, tot_e16], I16)
            nc.sync.dma_start(esrc_t[:], esrc_d[:])

            def build_ftab(l):
                """ftab[l] = h_l @ wcat[l]; layer 0 reads hT (pre-transposed),
                layer 1 PE-transposes hfull row chunks."""
                for k0 in range(0, NKCH, KC):
                    kn = min(KC, NKCH - k0)
                    stg = st.tile([128, KC, RFB], BF, tag="ftstg")
                    for kk in range(kn):
                        k = k0 + kk
                        rows = min(128, N - k * 128)
                        lhs = ld.tile([128, 128], BF, tag="lhs")
                        if l == 0:
                            if rows < 128:
                                nc.vector.memset(lhs[:], 0.0)
                            nc.sync.dma_start(lhs[:, :rows],
                                              hT_d[:, k * 128 : k * 128 + rows])
                        else:
                            hrow = ld.tile([128, 128], BF, tag="hrow")
                            if rows < 128:
                                nc.vector.memset(hrow[:], 0.0)
                            nc.sync.dma_start(
                                hrow[:rows, :],
                                hfull[k * 128 : k * 128 + rows, :])
                            ptr = psB.tile([128, 128], BF, tag="tr")
                            nc.tensor.transpose(ptr[:], hrow[:], ident_t[:])
                            nc.vector.tensor_copy(lhs[:], ptr[:])
                        pchunk = psA.tile([128, RFB], F32, tag="acc")
                        nc.tensor.matmul(pchunk[:], lhs[:], wcat_t[l][:],
                                         start=True, stop=True)
                        nc.vector.tensor_copy(stg[:, kk, :], pchunk[:])
                    dst = ftab[l][k0 * 128 : (k0 + kn) * 128, :]
                    nc.sync.dma_start(
                        dst.rearrange("(k p) r -> p k r", p=128), stg[:, :kn, :])

            def build_er(l):
                """er_all[l][:, g, :] = er for the core's dst shard."""
                for k in range(NG):
                    rows = min(128, NS - k * 128)
                    lhs = ld.tile([128, 128], BF, tag="lhs")
                    if l == 0:
                        if rows < 128:
                            nc.vector.memset(lhs[:], 0.0)
                        nc.sync.dma_start(lhs[:, :rows],
                                          hTs_d[:, k * 128 : k * 128 + rows])
                    else:
                        hrow = ld.tile([128, 128], BF, tag="hrow")
                        if rows < 128:
                            nc.vector.memset(hrow[:], 0.0)
                        nc.sync.dma_start(
                            hrow[:rows, :],
                            hshard[0][k * 128 : k * 128 + rows, :])
                        ptr = psB.tile([128, 128], BF, tag="tr")
                        nc.tensor.transpose(ptr[:], hrow[:], ident_t[:])
                        nc.vector.tensor_copy(lhs[:], ptr[:])
                    perp = psE.tile([128, 2], F32, tag="erp")
                    nc.tensor.matmul(perp[:], lhs[:], wer_t[l][:],
                                     start=True, stop=True)
                    nc.vector.tensor_copy(er_all[l][:, k, :], perp[:])

            def edge_phase(l):
                """One GAT aggregation layer; writes hshard[l]."""
                cc = 0  # global chunk cursor
                for g in range(NG):
                    gpsum = psA.tile([128, NAGG], F32, tag="acc")
                    nchunks_g = int(cpg[g].sum())
                    done = 0
                    for b in range(2):
                        base = b * HALF
                        nrows = (N - HALF) if b else HALF
                        nrem = int(cpg[g, b])
                        while nrem > 0:
                            n = min(RCAP, nrem)
                            nrem -= n
                            ne = n * 128
                            gt = ed.tile([128, RCAP, RFB], BF, tag="gt")
                            nc.gpsimd.dma_gather(
                                gt[:, :n, :], ftab[l][base : base + nrows, :],
                                esrc_t[:, cc * 8 : (cc + n) * 8], ne, ne, RFB)
                            # replicate dst ids across partitions for ohT
                            eb = ed.tile([128, RCAP, 128], BF, tag="eb")
                            nc.sync.dma_start(
                                eb[:, :n, :],
                                bass.AP(edstfl_d, cc * 128, [[0, 128], [1, ne]]))
                            # oh[e, c, d] = (iota[d] == dstf[e, c])
                            oh = ed.tile([128, RCAP, 128], BF, tag="oh")
                            i0 = iota_t[:]
                            iota_b = bass.AP(i0.tensor, i0.offset,
                                             [i0.ap[0], [0, n], [1, 128]])
                            e0 = edstf_t[:]
                            edstf_b = bass.AP(e0.tensor, e0.offset + cc,
                                              [e0.ap[0], [1, n], [0, 128]])
                            nc.vector.tensor_tensor(oh[:, :n, :], iota_b,
                                                    edstf_b, op=OP.is_equal)
                            # ohT[d, c, e] = (colv[d] == dstfB[d, c, e])
                            ohT = ed.tile([128, RCAP, 128], BF, tag="ohT")
                            c0 = colv_t[:]
                            colv_b = bass.AP(c0.tensor, c0.offset,
                                             [c0.ap[0], [0, n], [1, 128]])
                            nc.vector.tensor_tensor(ohT[:, :n, :], colv_b,
                                                    eb[:, :n, :],
                                                    op=OP.is_equal)
                            # er per edge: erps[:, 2c:2c+2] = ohT_c^T @ er_g
                            erps = psE.tile([128, 2 * RCAP], F32, tag="erp")
                            for i in range(n):
                                nc.tensor.matmul(erps[:, 2 * i : 2 * i + 2],
                                                 ohT[:, i, :],
                                                 er_all[l][:, g, :],
                                                 start=True, stop=True)
                            # s = exp(lrelu(el + er)); el at gt cols 256:258
                            s_t = sc.tile([128, RCAP, 2], F32, tag="s")
                            erv = erps[:]
                            er_ap = bass.AP(erv.tensor, erv.offset,
                                            [erv.ap[0], [2, n], [1, 2]])
                            nc.vector.tensor_tensor(
                                s_t[:, :n, :], gt[:, :n, 256:258], er_ap,
                                op=OP.add)
                            lr = sc.tile([128, RCAP, 2], F32, tag="lr")
                            nc.scalar.activation(lr[:, :n, :], s_t[:, :n, :],
                                                 AF.Relu, scale=1.0 - NEG)
                            nc.vector.tensor_scalar_mul(s_t[:, :n, :],
                                                        s_t[:, :n, :], NEG)
                            nc.vector.tensor_tensor(s_t[:, :n, :], s_t[:, :n, :],
                                                    lr[:, :n, :], op=OP.add)
                            nc.scalar.activation(gt[:, :n, 258:260],
                                                 s_t[:, :n, :], AF.Exp)
                            # msg = feat * s (broadcast over each head's cols)
                            g0 = gt[:]
                            feat_ap = bass.AP(
                                g0.tensor, g0.offset,
                                [g0.ap[0], [RFB, n], [128, 2], [1, 128]])
                            s_ap = bass.AP(
                                g0.tensor, g0.offset + 258,
                                [g0.ap[0], [RFB, n], [1, 2], [0, 128]])
                            nc.vector.tensor_tensor(feat_ap, feat_ap, s_ap,
                                                    op=OP.mult)
                            for i in range(n):
                                nc.tensor.matmul(gpsum[:], oh[:, i, :],
                                                 gt[:, i, 0:NAGG],
                                                 start=(done == 0),
                                                 stop=(done == nchunks_g - 1))
                                done += 1
                                cc += 1
                    # postprocess group -> h rows
                    den = sc.tile([128, 2], F32, tag="den")
                    nc.vector.tensor_scalar_add(den[:], gpsum[:, 258:260], 1e-9)
                    rec = sc.tile([128, 2], F32, tag="rec")
                    nc.vector.reciprocal(rec[:], den[:])
                    r0 = sc.tile([128, 128], F32, tag="r0")
                    nc.scalar.activation(r0[:], gpsum[:, 0:128], AF.Relu,
                                         scale=rec[:, 0:1])
                    r1 = sc.tile([128, 128], F32, tag="r1")
                    nc.scalar.activation(r1[:], gpsum[:, 128:256], AF.Relu,
                                         scale=rec[:, 1:2])
                    hsum = sc.tile([128, 128], F32, tag="hsum")
                    nc.vector.tensor_tensor(hsum[:], r0[:], r1[:], op=OP.add)
                    hrow = sc.tile([128, 128], BF, tag="hmean")
                    nc.vector.tensor_scalar_mul(hrow[:], hsum[:], 0.5)
                    rows = min(128, NS - g * 128)
                    nc.sync.dma_start(hshard[l][g * 128 : g * 128 + rows, :],
                                      hrow[:rows, :])

            def allgather(l):
                dst = hfull if l == 0 else h2full
                nc.gpsimd.collective_compute(
                    "AllGather", mybir.AluOpType.bypass,
                    ins=[hshard[l][:, :].opt()], outs=[dst[:, :].opt()],
                    replica_groups=[list(range(NCORES))])

            def head_phase():
                x1_t = cst.tile([128, hb * 8], I16, tag="x1i")
                nc.sync.dma_start(x1_t[:], x1_d[:])
                x2_t = cst.tile([128, hb * 8], I16, tag="x2i")
                nc.sync.dma_start(x2_t[:], x2_d[:])
                outsb = cst.tile([C, TOTP], F32, tag="outsb")
                hc = 0
                for q in range(4):
                    base1 = (q & 1) * HALF
                    base2 = ((q >> 1) & 1) * HALF
                    nr1 = (N - HALF) if (q & 1) else HALF
                    nr2 = (N - HALF) if ((q >> 1) & 1) else HALF
                    nrem = int(pb[q])
                    while nrem > 0:
                        n = min(RCAP, nrem)
                        nrem -= n
                        ne = n * 128
                        g1 = hgp.tile([128, RCAP, 128], BF, tag="hg1")
                        g2 = hgp.tile([128, RCAP, 128], BF, tag="hg2")
                        nc.gpsimd.dma_gather(
                            g1[:, :n, :], h2full[base1 : base1 + nr1, :],
                            x1_t[:, hc * 8 : (hc + n) * 8], ne, ne, 128)
                        nc.gpsimd.dma_gather(
                            g2[:, :n, :], h2full[base2 : base2 + nr2, :],
                            x2_t[:, hc * 8 : (hc + n) * 8], ne, ne, 128)
                        dt_ = hgp.tile([128, RCAP, 128], BF, tag="hd")
                        nc.vector.tensor_tensor(dt_[:, :n, :], g1[:, :n, :],
                                                g2[:, :n, :], op=OP.subtract)
                        nc.scalar.activation(dt_[:, :n, :], dt_[:, :n, :],
                                             AF.Abs)
                        for i in range(n):
                            po1 = psA.tile([128, 128], F32, tag="acc")
                            for j, tsrc in enumerate((g1, g2, dt_)):
                                ptr = psB.tile([128, 128], BF, tag="tr")
                                nc.tensor.transpose(ptr[:], tsrc[:, i, :],
                                                    ident_t[:])
                                tsb = sc.tile([128, 128], BF, tag="htsb")
                                nc.vector.tensor_copy(tsb[:], ptr[:])
                                nc.tensor.matmul(po1[:], w1_t[:, j, :], tsb[:],
                                                 start=(j == 0), stop=(j == 2))
                            o1 = sc.tile([128, 128], BF, tag="ho1")
                            nc.scalar.activation(o1[:], po1[:], AF.Relu,
                                                 bias=b1_t[:, 0:1])
                            po2 = psA.tile([C, 128], F32, tag="acc")
                            nc.tensor.matmul(po2[:], w2_t[:], o1[:],
                                             start=True, stop=True)
                            nc.vector.tensor_scalar(
                                outsb[:, hc * 128 : (hc + 1) * 128], po2[:],
                                b2_t[:, 0:1], None, OP.add)
                            hc += 1
                nc.sync.dma_start(out_d[:], outsb[:])

            build_ftab(0)
            build_er(0)
            edge_phase(0)
            allgather(0)
            build_ftab(1)
            build_er(1)
            edge_phase(1)
            allgather(1)
            head_phase()

    nc.compile()
    return nc


def _prepare_inputs(src, dst, h, x1, x2, W0, al0, ar0, W1, al1, ar1,
                    w1, b1, w2, b2):
    cpg, ecores = _build_edge_schedule(src, dst)
    pb, hcores = _build_head_schedule(x1, x2)

    Wcat0, Wer0 = _fold_weights(np.asarray(W0, np.float32),
                                np.asarray(al0, np.float32),
                                np.asarray(ar0, np.float32))
    Wcat1, Wer1 = _fold_weights(np.asarray(W1, np.float32),
                                np.asarray(al1, np.float32),
                                np.asarray(ar1, np.float32))
    hT = np.ascontiguousarray(np.asarray(h, np.float32).T).astype(BF16)
    iota = np.tile(np.arange(128, dtype=np.float32), (128, 1)).astype(BF16)
    colv = np.tile(np.arange(128, dtype=np.float32)[:, None],
                   (1, 128)).astype(BF16)
    ident = np.eye(128, dtype=np.float32).astype(BF16)

    in_maps = []
    for c in range(NCORES):
        ec, hcj = ecores[c], hcores[c]
        in_maps.append({
            "hT": hT,
            "hTs": np.ascontiguousarray(hT[:, c * NS : (c + 1) * NS]),
            "wcat0": Wcat0, "wer0": Wer0, "wcat1": Wcat1, "wer1": Wer1,
            "w1": np.asarray(w1, np.float32).astype(BF16),
            "b1": np.asarray(b1, np.float32).reshape(NH, 1),
            "w2": np.asarray(w2, np.float32).astype(BF16),
            "b2": np.asarray(b2, np.float32).reshape(C, 1),
            "iota": iota, "colv": colv, "ident": ident,
            "esrc": ec["esrc16"], "edstf": ec["edstf"],
            "edstfl": ec["edstf_flat"],
            "x1i": hcj["x1_16"], "x2i": hcj["x2_16"],
        })
    return cpg, pb, in_maps, hcores


def kernel(src, dst, h, x1, x2, W0, al0, ar0, W1, al1, ar1, w1, b1, w2, b2):
    src = np.asarray(src, np.int64)
    dst = np.asarray(dst, np.int64)
    x1 = np.asarray(x1, np.int64)
    x2 = np.asarray(x2, np.int64)

    cpg, pb, in_maps, hcores = _prepare_inputs(
        src, dst, h, x1, x2, W0, al0, ar0, W1, al1, ar1, w1, b1, w2, b2)

    key = (cpg.tobytes(), pb.tobytes())
    if key not in _CACHE:
        _CACHE.clear()
        _CACHE[key] = _build_program(cpg, pb)
    nc = _CACHE[key]

    from concourse.bass_utils import run_bass_kernel_spmd
    kw = {"trace": True} if _TRACE else {}
    res = run_bass_kernel_spmd(nc, in_maps, core_ids=list(range(NCORES)), **kw)
    global LAST_PERF
    LAST_PERF = res

    PC = P // NCORES
    out = np.empty((P, C), np.float32)
    for c in range(NCORES):
        cols = res.results[c]["headout"]          # [C, TOTP]
        out[c * PC : (c + 1) * PC, :] = cols[:, hcores[c]["posmap"]].T
    return out


# revision 12
# speedup vs baseline: 1.1213x; 1.1213x over previous
"""Bass/Trainium2 kernel for the 2-layer GAT + pair-MLP problem.

Self-contained: builds an edge/pair schedule from the actual inputs,
compiles one SPMD Bass program, runs it on 8 NeuronCores, reassembles the
full [P, 2] output.

Sharding: nodes are split into 8 contiguous shards (dst-owner cores);
each core aggregates all edges whose destination lies in its shard, with
the full per-layer feature table replicated in its HBM (fp16). The pair
batch is split 1/8 per core. Inter-layer node features are exchanged with
an on-device AllGather.

Per GAT layer (all on-device, fp16 data / f32 accumulation):
  ftab = h @ [W | W@al_blockdiag | 0pad]     # [N,384] fp16: feat(256)+el(2)+pad
  er_all = h_shard @ (W@ar_blockdiag)        # [128, NG, 2] resident in SBUF
  per edge run (<=RCAP chunks of 128 edges, same 128-dst group, same src half):
    dma_gather ftab rows by src (the only per-edge gather)
    one-hot oh[e,d] and its transpose ohT[d,e] built as single broadcast
      is_equal ops (fp16); er per edge = ohT-matmul against er_all[:,g,:]
    s = exp(leaky_relu(el+er)) in f32, written into row cols 258:260 (fp16)
    msg = feat * s (per-head broadcast, fp16)
    oh matmul accumulates [128dst, 260] into PSUM (f32)
  out[d,h] = relu(acc_h * recip(den_h + 1e-9)); h[d] = mean over heads (fp16)
"""
import numpy as np
BF16 = np.float16  # 2-byte element dtype for tables/messages

# problem constants (shapes the grader feeds)
N = 50000
E = 800000
IN = 128
NH = 128
H = 2
P = 100000
C = 2
NEG = 0.2

NCORES = 8
NS = N // NCORES          # nodes per shard
GRP = 128                 # dst group size (one PSUM accumulation)
HALF = 32768              # int16 gather index range split
RFB = 384                 # ftab row fp16 cols: 256 feat + 2 el + 2 s + pad
NAGG = 260                # aggregated row cols: 256 msg + 2 el(dead) + 2 s
PADDST = 200.0            # one-hot miss marker for padded edges
RCAP = 8                  # max chunks per gather sub-run (SBUF bound)

_CACHE = {}
_TRACE = False      # set by test harnesses to capture an NTFF profile
LAST_PERF = None


def _ng():
    return -(-NS // GRP)


def _ceil_div(a, b):
    return -(-a // b)


def _wrap_idx16(idx):
    """int16 idx j -> [(j%16) (+16k replicas), j//16] as [128, n/16]."""
    idx = np.asarray(idx, np.int16)
    assert idx.size % 16 == 0
    w = idx.reshape(-1, 16).T
    return np.tile(w, (8, 1)).copy()


def _fold_weights(W, al, ar):
    """Wcat [d, RFB] = [W | W@al_blk | 0] (fp16); Wer [d, 2] = W@ar_blk."""
    d = W.shape[0]
    Wcat = np.zeros((d, RFB), np.float32)
    Wer = np.zeros((d, 2), np.float32)
    Wcat[:, : H * NH] = W
    for h in range(H):
        Wcat[:, H * NH + h] = W[:, h * NH : (h + 1) * NH] @ al[h]
        Wer[:, h] = W[:, h * NH : (h + 1) * NH] @ ar[h]
    return Wcat.astype(BF16), Wer.astype(BF16)


def _build_edge_schedule(src, dst):
    """Uniform (group, bucket) chunk schedule shared by all cores."""
    NG = _ng()
    src = np.asarray(src, np.int64)
    dst = np.asarray(dst, np.int64)
    core = dst // NS
    dloc = dst - core * NS
    g = dloc // GRP
    b = (src >= HALF).astype(np.int64)
    key = (core * NG + g) * 2 + b
    counts = np.bincount(key, minlength=NCORES * NG * 2).reshape(NCORES, NG, 2)
    cpg = np.maximum(1, _ceil_div(counts.max(axis=0), 128))  # [NG, 2]
    run_chunk_start = np.concatenate([[0], np.cumsum(cpg.reshape(-1))])
    tot_chunks = int(run_chunk_start[-1])
    tot_edges = tot_chunks * 128

    order = np.argsort(key, kind="stable")
    key_s = key[order]
    seg_start = np.searchsorted(key_s, np.arange(NCORES * NG * 2))
    rank = np.arange(src.size) - seg_start[key_s]
    gb = key_s % (NG * 2)
    pos = run_chunk_start[gb] * 128 + rank
    core_s = key_s // (NG * 2)

    cores = []
    for c in range(NCORES):
        m = core_s == c
        e = order[m]
        p = pos[m]
        esrc16 = np.zeros(tot_edges, np.int16)
        edstf = np.full(tot_edges, PADDST, np.float32)
        esrc16[p] = (src[e] - b[e] * HALF).astype(np.int16)
        edstf[p] = (dloc[e] % GRP).astype(np.float32)
        edstf = edstf.astype(BF16)
        cores.append(dict(
            esrc16=_wrap_idx16(esrc16),
            edstf=edstf.reshape(tot_chunks, 128).T.copy(),
            edstf_flat=edstf.reshape(1, tot_edges).copy(),
        ))
    return cpg, cores


def _build_head_schedule(x1, x2):
    """4 buckets by (x1>=HALF, x2>=HALF); uniform chunk counts across cores."""
    x1 = np.asarray(x1, np.int64)
    x2 = np.asarray(x2, np.int64)
    PC = P // NCORES
    q_all = (x1 >= HALF).astype(np.int64) + 2 * (x2 >= HALF).astype(np.int64)
    pb = np.ones(4, np.int64)
    for c in range(NCORES):
        cnt = np.bincount(q_all[c * PC : (c + 1) * PC], minlength=4)
        pb = np.maximum(pb, _ceil_div(cnt, 128))
    starts = np.concatenate([[0], np.cumsum(pb)]) * 128
    tot = int(starts[-1])
    percore = []
    for c in range(NCORES):
        sl = slice(c * PC, (c + 1) * PC)
        q = q_all[sl]
        x1c, x2c = x1[sl], x2[sl]
        x1_16 = np.zeros(tot, np.int16)
        x2_16 = np.zeros(tot, np.int16)
        posmap = np.zeros(PC, np.int64)
        for qq in range(4):
            m = q == qq
            n = int(m.sum())
            p = starts[qq] + np.arange(n)
            x1_16[p] = (x1c[m] - (qq & 1) * HALF).astype(np.int16)
            x2_16[p] = (x2c[m] - ((qq >> 1) & 1) * HALF).astype(np.int16)
            posmap[np.nonzero(m)[0]] = p
        percore.append(dict(
            x1_16=_wrap_idx16(x1_16),
            x2_16=_wrap_idx16(x2_16),
            posmap=posmap,
        ))
    return pb, percore


def _build_program(cpg, pb):
    import concourse.bacc as bacc
    import concourse.bass as bass
    import concourse.mybir as mybir
    import concourse.tile as tile

    F32 = mybir.dt.float32
    BF = mybir.dt.float16
    I16 = mybir.dt.int16
    AF = mybir.ActivationFunctionType
    OP = mybir.AluOpType

    NG = _ng()
    NKCH = _ceil_div(N, 128)        # node chunks for ftab builds
    NPAD = NKCH * 128
    tot_chunks = int(cpg.sum())
    tot_e16 = tot_chunks * 8        # idx cols (128 edges -> 8 cols of 16)
    tot_edges = tot_chunks * 128
    hb = int(pb.sum())              # head chunks per core
    TOTP = hb * 128
    KC = 8                          # ftab staging chunks per DMA

    nc = bacc.Bacc("TRN2", target_bir_lowering=False, debug=False,
                   num_devices=NCORES, num_swdge_queues=4)

    # ---- I/O ----
    hT_d = nc.dram_tensor("hT", [128, N], BF, kind="ExternalInput")
    hTs_d = nc.dram_tensor("hTs", [128, NS], BF, kind="ExternalInput")
    wcat_d = [nc.dram_tensor(f"wcat{l}", [128, RFB], BF, kind="ExternalInput")
              for l in range(2)]
    wer_d = [nc.dram_tensor(f"wer{l}", [128, 2], BF, kind="ExternalInput")
             for l in range(2)]
    w1_d = nc.dram_tensor("w1", [3 * NH, NH], BF, kind="ExternalInput")
    b1_d = nc.dram_tensor("b1", [NH, 1], F32, kind="ExternalInput")
    w2_d = nc.dram_tensor("w2", [NH, C], BF, kind="ExternalInput")
    b2_d = nc.dram_tensor("b2", [C, 1], F32, kind="ExternalInput")
    iota_d = nc.dram_tensor("iota", [128, RCAP * 128], BF, kind="ExternalInput")
    colv_d = nc.dram_tensor("colv", [128, RCAP * 128], BF, kind="ExternalInput")
    ident_d = nc.dram_tensor("ident", [128, 128], BF, kind="ExternalInput")
    esrc_d = nc.dram_tensor("esrc", [128, tot_e16], I16, kind="ExternalInput")
    edstf_d = nc.dram_tensor("edstf", [128, tot_chunks], BF,
                             kind="ExternalInput")
    edstfl_d = nc.dram_tensor("edstfl", [1, tot_edges], BF,
                              kind="ExternalInput")
    x1_d = nc.dram_tensor("x1i", [128, hb * 8], I16, kind="ExternalInput")
    x2_d = nc.dram_tensor("x2i", [128, hb * 8], I16, kind="ExternalInput")
    out_d = nc.dram_tensor("headout", [C, TOTP], F32, kind="ExternalOutput")

    with tile.TileContext(nc) as tc:
        with (
            tc.tile_pool(name="dram", bufs=1, space="DRAM") as dp,
            tc.tile_pool(name="cst", bufs=1) as cst,
            tc.tile_pool(name="ld", bufs=3) as ld,
            tc.tile_pool(name="st", bufs=2) as st,
            tc.tile_pool(name="ed", bufs=3) as ed,
            tc.tile_pool(name="hg", bufs=2) as hgp,
            tc.tile_pool(name="sc", bufs=4) as sc,
            tc.tile_pool(name="psA", bufs=3, space="PSUM") as psA,
            tc.tile_pool(name="psB", bufs=2, space="PSUM") as psB,
            tc.tile_pool(name="psE", bufs=2, space="PSUM") as psE,
        ):
            # ---- internal DRAM (pool tiles so Tile tracks RAW deps) ----
            ftab = [dp.tile([NPAD, RFB], BF, name=f"ftab{l}") for l in range(2)]
            hshard = [dp.tile([NS, 128], BF, name=f"hshard{l}")
                      for l in range(2)]
            hfull = dp.tile([N, 128], BF, name="hfull", addr_space="Shared")
            h2full = dp.tile([N, 128], BF, name="h2full", addr_space="Shared")

            # ---- constants ----
            iota_t = cst.tile([128, RCAP, 128], BF)
            nc.sync.dma_start(iota_t[:], iota_d[:].rearrange("p (c x) -> p c x", x=128))
            colv_t = cst.tile([128, RCAP, 128], BF)
            nc.sync.dma_start(colv_t[:], colv_d[:].rearrange("p (c x) -> p c x", x=128))
            ident_t = cst.tile([128, 128], BF)
            nc.sync.dma_start(ident_t[:], ident_d[:])
            wcat_t, wer_t = [], []
            for l in range(2):
                w = cst.tile([128, RFB], BF, name=f"wcat{l}t")
                nc.sync.dma_start(w[:], wcat_d[l][:])
                wcat_t.append(w)
                w = cst.tile([128, 2], BF, name=f"wer{l}t")
                nc.sync.dma_start(w[:], wer_d[l][:])
                wer_t.append(w)
            er_all = [cst.tile([128, NG, 2], BF, name=f"erall{l}")
                      for l in range(2)]
            w1_t = cst.tile([128, 3, NH], BF)
            for j in range(3):
                nc.sync.dma_start(w1_t[:, j, :], w1_d[j * 128 : (j + 1) * 128, :])
            b1_t = cst.tile([NH, 1], F32)
            nc.sync.dma_start(b1_t[:], b1_d[:])
            w2_t = cst.tile([NH, C], BF)
            nc.sync.dma_start(w2_t[:], w2_d[:])
            b2_t = cst.tile([C, 1], F32)
            nc.sync.dma_start(b2_t[:], b2_d[:])
            edstf_t = cst.tile([128, tot_chunks], BF)
            nc.sync.dma_start(edstf_t[:], edstf_d[:])
            esrc_t = cst.tile([128, tot_e16], I16)
            nc.sync.dma_start(esrc_t[:], esrc_d[:])

            qn = [0]  # SWDGE queue round-robin cursor

            def build_ftab(l):
                """ftab[l] = h_l @ wcat[l]; layer 0 reads hT (pre-transposed),
                layer 1 PE-transposes hfull row chunks."""
                for k0 in range(0, NKCH, KC):
                    kn = min(KC, NKCH - k0)
                    stg = st.tile([128, KC, RFB], BF, tag="ftstg")
                    for kk in range(kn):
                        k = k0 + kk
                        rows = min(128, N - k * 128)
                        lhs = ld.tile([128, 128], BF, tag="lhs")
                        if l == 0:
                            if rows < 128:
                                nc.vector.memset(lhs[:], 0.0)
                            nc.sync.dma_start(lhs[:, :rows],
                                              hT_d[:, k * 128 : k * 128 + rows])
                        else:
                            hrow = ld.tile([128, 128], BF, tag="hrow")
                            if rows < 128:
                                nc.vector.memset(hrow[:], 0.0)
                            nc.sync.dma_start(
                                hrow[:rows, :],
                                hfull[k * 128 : k * 128 + rows, :])
                            ptr = psB.tile([128, 128], BF, tag="tr")
                            nc.tensor.transpose(ptr[:], hrow[:], ident_t[:])
                            nc.vector.tensor_copy(lhs[:], ptr[:])
                        pchunk = psA.tile([128, RFB], F32, tag="acc")
                        nc.tensor.matmul(pchunk[:], lhs[:], wcat_t[l][:],
                                         start=True, stop=True)
                        nc.scalar.activation(stg[:, kk, :], pchunk[:], AF.Copy)
                    dst = ftab[l][k0 * 128 : (k0 + kn) * 128, :]
                    nc.sync.dma_start(
                        dst.rearrange("(k p) r -> p k r", p=128), stg[:, :kn, :])

            def build_er(l):
                """er_all[l][:, g, :] = er for the core's dst shard."""
                for k in range(NG):
                    rows = min(128, NS - k * 128)
                    lhs = ld.tile([128, 128], BF, tag="lhs")
                    if l == 0:
                        if rows < 128:
                            nc.vector.memset(lhs[:], 0.0)
                        nc.sync.dma_start(lhs[:, :rows],
                                          hTs_d[:, k * 128 : k * 128 + rows])
                    else:
                        hrow = ld.tile([128, 128], BF, tag="hrow")
                        if rows < 128:
                            nc.vector.memset(hrow[:], 0.0)
                        nc.sync.dma_start(
                            hrow[:rows, :],
                            hshard[0][k * 128 : k * 128 + rows, :])
                        ptr = psB.tile([128, 128], BF, tag="tr")
                        nc.tensor.transpose(ptr[:], hrow[:], ident_t[:])
                        nc.vector.tensor_copy(lhs[:], ptr[:])
                    perp = psE.tile([128, 2], F32, tag="erp")
                    nc.tensor.matmul(perp[:], lhs[:], wer_t[l][:],
                                     start=True, stop=True)
                    nc.vector.tensor_copy(er_all[l][:, k, :], perp[:])

            def edge_phase(l):
                """One GAT aggregation layer; writes hshard[l]."""
                cc = 0  # global chunk cursor
                for g in range(NG):
                    gpsum = psA.tile([128, NAGG], F32, tag="acc")
                    nchunks_g = int(cpg[g].sum())
                    done = 0
                    for b in range(2):
                        base = b * HALF
                        nrows = (N - HALF) if b else HALF
                        nrem = int(cpg[g, b])
                        while nrem > 0:
                            n = min(RCAP, nrem)
                            nrem -= n
                            ne = n * 128
                            gt = ed.tile([128, RCAP, RFB], BF, tag="gt")
                            nc.gpsimd.dma_gather(
                                gt[:, :n, :], ftab[l][base : base + nrows, :],
                                esrc_t[:, cc * 8 : (cc + n) * 8], ne, ne, RFB,
                                queue_num=qn[0] % 4)
                            qn[0] += 1
                            # replicate dst ids across partitions for ohT
                            eb = ed.tile([128, RCAP, 128], BF, tag="eb")
                            nc.sync.dma_start(
                                eb[:, :n, :],
                                bass.AP(edstfl_d, cc * 128, [[0, 128], [1, ne]]))
                            # oh[e, c, d] = (iota[d] == dstf[e, c])
                            oh = ed.tile([128, RCAP, 128], BF, tag="oh")
                            e0 = edstf_t[:]
                            edstf_b = bass.AP(e0.tensor, e0.offset + cc,
                                              [e0.ap[0], [1, n], [0, 128]])
                            nc.vector.tensor_tensor(oh[:, :n, :],
                                                    iota_t[:, :n, :],
                                                    edstf_b, op=OP.is_equal)
                            # ohT[d, c, e] = (colv[d] == dstfB[d, c, e])
                            ohT = ed.tile([128, RCAP, 128], BF, tag="ohT")
                            nc.vector.tensor_tensor(ohT[:, :n, :],
                                                    colv_t[:, :n, :],
                                                    eb[:, :n, :],
                                                    op=OP.is_equal)
                            # er per edge: erps[:, 2c:2c+2] = ohT_c^T @ er_g
                            erps = psE.tile([128, 2 * RCAP], F32, tag="erp")
                            for i in range(n):
                                nc.tensor.matmul(erps[:, 2 * i : 2 * i + 2],
                                                 ohT[:, i, :],
                                                 er_all[l][:, g, :],
                                                 start=True, stop=True)
                            # s = exp(lrelu(el + er)); el at gt cols 256:258
                            s_t = sc.tile([128, RCAP, 2], F32, tag="s")
                            erv = erps[:]
                            er_ap = bass.AP(erv.tensor, erv.offset,
                                            [erv.ap[0], [2, n], [1, 2]])
                            nc.vector.tensor_tensor(
                                s_t[:, :n, :], gt[:, :n, 256:258], er_ap,
                                op=OP.add)
                            lr = sc.tile([128, RCAP, 2], F32, tag="lr")
                            nc.scalar.activation(lr[:, :n, :], s_t[:, :n, :],
                                                 AF.Exp)
                            nc.scalar.activation(s_t[:, :n, :], s_t[:, :n, :],
                                                 AF.Exp, scale=NEG)
                            nc.vector.tensor_tensor(gt[:, :n, 258:260],
                                                    lr[:, :n, :], s_t[:, :n, :],
                                                    op=OP.max)
                            # msg = feat * s (broadcast over each head's cols)
                            g0 = gt[:]
                            feat_ap = bass.AP(
                                g0.tensor, g0.offset,
                                [g0.ap[0], [RFB, n], [128, 2], [1, 128]])
                            s_ap = bass.AP(
                                g0.tensor, g0.offset + 258,
                                [g0.ap[0], [RFB, n], [1, 2], [0, 128]])
                            nc.vector.tensor_tensor(feat_ap, feat_ap, s_ap,
                                                    op=OP.mult)
                            for i in range(n):
                                nc.tensor.matmul(gpsum[:], oh[:, i, :],
                                                 gt[:, i, 0:NAGG],
                                                 start=(done == 0),
                                                 stop=(done == nchunks_g - 1))
                                done += 1
                                cc += 1
                    # postprocess group -> h rows
                    den = sc.tile([128, 2], F32, tag="den")
                    nc.vector.tensor_scalar_add(den[:], gpsum[:, 258:260], 1e-9)
                    rec = sc.tile([128, 2], F32, tag="rec")
                    nc.vector.reciprocal(rec[:], den[:])
                    r0 = sc.tile([128, 128], F32, tag="r0")
                    nc.scalar.activation(r0[:], gpsum[:, 0:128], AF.Relu,
                                         scale=rec[:, 0:1])
                    r1 = sc.tile([128, 128], F32, tag="r1")
                    nc.scalar.activation(r1[:], gpsum[:, 128:256], AF.Relu,
                                         scale=rec[:, 1:2])
                    hsum = sc.tile([128, 128], F32, tag="hsum")
                    nc.vector.tensor_tensor(hsum[:], r0[:], r1[:], op=OP.add)
                    hrow = sc.tile([128, 128], BF, tag="hmean")
                    nc.scalar.activation(hrow[:], hsum[:], AF.Copy, scale=0.5)
                    rows = min(128, NS - g * 128)
                    nc.sync.dma_start(hshard[l][g * 128 : g * 128 + rows, :],
                                      hrow[:rows, :])

            def allgather(l):
                dst = hfull if l == 0 else h2full
                nc.gpsimd.collective_compute(
                    "AllGather", mybir.AluOpType.bypass,
                    ins=[hshard[l][:, :].opt()], outs=[dst[:, :].opt()],
                    replica_groups=[list(range(NCORES))])

            def head_phase():
                x1_t = cst.tile([128, hb * 8], I16, tag="x1i")
                nc.sync.dma_start(x1_t[:], x1_d[:])
                x2_t = cst.tile([128, hb * 8], I16, tag="x2i")
                nc.sync.dma_start(x2_t[:], x2_d[:])
                outsb = cst.tile([C, TOTP], F32, tag="outsb")
                hc = 0
                for q in range(4):
                    base1 = (q & 1) * HALF
                    base2 = ((q >> 1) & 1) * HALF
                    nr1 = (N - HALF) if (q & 1) else HALF
                    nr2 = (N - HALF) if ((q >> 1) & 1) else HALF
                    nrem = int(pb[q])
                    while nrem > 0:
                        n = min(RCAP, nrem)
                        nrem -= n
                        ne = n * 128
                        g1 = hgp.tile([128, RCAP, 128], BF, tag="hg1")
                        g2 = hgp.tile([128, RCAP, 128], BF, tag="hg2")
                        nc.gpsimd.dma_gather(
                            g1[:, :n, :], h2full[base1 : base1 + nr1, :],
                            x1_t[:, hc * 8 : (hc + n) * 8], ne, ne, 128,
                            queue_num=qn[0] % 4)
                        qn[0] += 1
                        nc.gpsimd.dma_gather(
                            g2[:, :n, :], h2full[base2 : base2 + nr2, :],
                            x2_t[:, hc * 8 : (hc + n) * 8], ne, ne, 128,
                            queue_num=qn[0] % 4)
                        qn[0] += 1
                        dt_ = hgp.tile([128, RCAP, 128], BF, tag="hd")
                        nc.vector.tensor_tensor(dt_[:, :n, :], g1[:, :n, :],
                                                g2[:, :n, :], op=OP.subtract)
                        nc.scalar.activation(dt_[:, :n, :], dt_[:, :n, :],
                                             AF.Abs)
                        for i in range(n):
                            po1 = psA.tile([128, 128], F32, tag="acc")
                            for j, tsrc in enumerate((g1, g2, dt_)):
                                ptr = psB.tile([128, 128], BF, tag="tr")
                                nc.tensor.transpose(ptr[:], tsrc[:, i, :],
                                                    ident_t[:])
                                tsb = sc.tile([128, 128], BF, tag="htsb")
                                nc.vector.tensor_copy(tsb[:], ptr[:])
                                nc.tensor.matmul(po1[:], w1_t[:, j, :], tsb[:],
                                                 start=(j == 0), stop=(j == 2))
                            o1 = sc.tile([128, 128], BF, tag="ho1")
                            nc.scalar.activation(o1[:], po1[:], AF.Relu,
                                                 bias=b1_t[:, 0:1])
                            po2 = psA.tile([C, 128], F32, tag="acc")
                            nc.tensor.matmul(po2[:], w2_t[:], o1[:],
                                             start=True, stop=True)
                            nc.vector.tensor_scalar(
                                outsb[:, hc * 128 : (hc + 1) * 128], po2[:],
                                b2_t[:, 0:1], None, OP.add)
                            hc += 1
                nc.sync.dma_start(out_d[:], outsb[:])

            build_ftab(0)
            build_er(0)
            edge_phase(0)
            allgather(0)
            build_ftab(1)
            build_er(1)
            edge_phase(1)
            allgather(1)
            head_phase()

    nc.compile()
    return nc


def _prepare_inputs(src, dst, h, x1, x2, W0, al0, ar0, W1, al1, ar1,
                    w1, b1, w2, b2):
    cpg, ecores = _build_edge_schedule(src, dst)
    pb, hcores = _build_head_schedule(x1, x2)

    Wcat0, Wer0 = _fold_weights(np.asarray(W0, np.float32),
                                np.asarray(al0, np.float32),
                                np.asarray(ar0, np.float32))
    Wcat1, Wer1 = _fold_weights(np.asarray(W1, np.float32),
                                np.asarray(al1, np.float32),
                                np.asarray(ar1, np.float32))
    hT = np.ascontiguousarray(np.asarray(h, np.float32).T).astype(BF16)
    iota = np.tile(np.arange(128, dtype=np.float32), (128, RCAP)).astype(BF16)
    colv = np.tile(np.arange(128, dtype=np.float32)[:, None],
                   (1, RCAP * 128)).astype(BF16)
    ident = np.eye(128, dtype=np.float32).astype(BF16)

    in_maps = []
    for c in range(NCORES):
        ec, hcj = ecores[c], hcores[c]
        in_maps.append({
            "hT": hT,
            "hTs": np.ascontiguousarray(hT[:, c * NS : (c + 1) * NS]),
            "wcat0": Wcat0, "wer0": Wer0, "wcat1": Wcat1, "wer1": Wer1,
            "w1": np.asarray(w1, np.float32).astype(BF16),
            "b1": np.asarray(b1, np.float32).reshape(NH, 1),
            "w2": np.asarray(w2, np.float32).astype(BF16),
            "b2": np.asarray(b2, np.float32).reshape(C, 1),
            "iota": iota, "colv": colv, "ident": ident,
            "esrc": ec["esrc16"], "edstf": ec["edstf"],
            "edstfl": ec["edstf_flat"],
            "x1i": hcj["x1_16"], "x2i": hcj["x2_16"],
        })
    return cpg, pb, in_maps, hcores


def kernel(src, dst, h, x1, x2, W0, al0, ar0, W1, al1, ar1, w1, b1, w2, b2):
    src = np.asarray(src, np.int64)
    dst = np.asarray(dst, np.int64)
    x1 = np.asarray(x1, np.int64)
    x2 = np.asarray(x2, np.int64)

    cpg, pb, in_maps, hcores = _prepare_inputs(
        src, dst, h, x1, x2, W0, al0, ar0, W1, al1, ar1, w1, b1, w2, b2)

    key = (cpg.tobytes(), pb.tobytes())
    if key not in _CACHE:
        _CACHE.clear()
        _CACHE[key] = _build_program(cpg, pb)
    nc = _CACHE[key]

    from concourse.bass_utils import run_bass_kernel_spmd
    kw = {"trace": True} if _TRACE else {}
    res = run_bass_kernel_spmd(nc, in_maps, core_ids=list(range(NCORES)), **kw)
    global LAST_PERF
    LAST_PERF = res

    PC = P // NCORES
    out = np.empty((P, C), np.float32)
    for c in range(NCORES):
        cols = res.results[c]["headout"]          # [C, TOTP]
        out[c * PC : (c + 1) * PC, :] = cols[:, hcores[c]["posmap"]].T
    return out


# revision 14
# speedup vs baseline: 1.3358x; 1.1913x over previous
"""Bass/Trainium2 kernel for the 2-layer GAT + pair-MLP problem.

Self-contained: builds an edge/pair schedule from the actual inputs,
compiles one SPMD Bass program, runs it on 8 NeuronCores, reassembles the
full [P, 2] output.

Sharding: nodes are split into 8 contiguous shards (dst-owner cores);
each core aggregates all edges whose destination lies in its shard, with
the full per-layer feature table replicated in its HBM (fp16). The pair
batch is split 1/8 per core. Inter-layer node features are exchanged with
an on-device AllGather.

Per GAT layer (all on-device, fp16 data / f32 accumulation):
  ftab = h @ [W | W@al_blockdiag | 0pad]     # [N,384] fp16: feat(256)+el(2)+pad
  er_all = h_shard @ (W@ar_blockdiag)        # [128, NG, 2] resident in SBUF
  per edge run (<=RCAP chunks of 128 edges, same 128-dst group, same src half):
    dma_gather ftab rows by src (the only per-edge gather)
    one-hot oh[e,d] and its transpose ohT[d,e] built as single broadcast
      is_equal ops (fp16); er per edge = ohT-matmul against er_all[:,g,:]
    s = exp(leaky_relu(el+er)) in f32, written into row cols 258:260 (fp16)
    msg = feat * s (per-head broadcast, fp16)
    oh matmul accumulates [128dst, 260] into PSUM (f32)
  out[d,h] = relu(acc_h * recip(den_h + 1e-9)); h[d] = mean over heads (fp16)
"""
import numpy as np
BF16 = np.float16  # 2-byte element dtype for tables/messages

# problem constants (shapes the grader feeds)
N = 50000
E = 800000
IN = 128
NH = 128
H = 2
P = 100000
C = 2
NEG = 0.2

NCORES = 8
NS = N // NCORES          # nodes per shard
GRP = 128                 # dst group size (one PSUM accumulation)
HALF = 32768              # int16 gather index range split
RFB = 384                 # ftab row fp16 cols: 256 feat + 2 el + 2 s + pad
NAGG = 260                # aggregated row cols: 256 msg + 2 el(dead) + 2 s
PADDST = 200.0            # one-hot miss marker for padded edges
RCAP = 8                  # max chunks per gather sub-run (SBUF bound)

_CACHE = {}
_TRACE = False      # set by test harnesses to capture an NTFF profile
LAST_PERF = None


def _ng():
    return -(-NS // GRP)


def _ceil_div(a, b):
    return -(-a // b)


def _wrap_idx16(idx):
    """int16 idx j -> [(j%16) (+16k replicas), j//16] as [128, n/16]."""
    idx = np.asarray(idx, np.int16)
    assert idx.size % 16 == 0
    w = idx.reshape(-1, 16).T
    return np.tile(w, (8, 1)).copy()


def _fold_weights(W, al, ar):
    """Wcat [d, RFB] = [W | W@al_blk | 0] (fp16); Wer [d, 2] = W@ar_blk."""
    d = W.shape[0]
    Wcat = np.zeros((d, RFB), np.float32)
    Wer = np.zeros((d, 2), np.float32)
    Wcat[:, : H * NH] = W
    for h in range(H):
        Wcat[:, H * NH + h] = W[:, h * NH : (h + 1) * NH] @ al[h]
        Wer[:, h] = W[:, h * NH : (h + 1) * NH] @ ar[h]
    return Wcat.astype(BF16), Wer.astype(BF16)


def _build_edge_schedule(src, dst):
    """Uniform (group, bucket) chunk schedule shared by all cores."""
    NG = _ng()
    src = np.asarray(src, np.int64)
    dst = np.asarray(dst, np.int64)
    core = dst // NS
    dloc = dst - core * NS
    g = dloc // GRP
    b = (src >= HALF).astype(np.int64)
    key = (core * NG + g) * 2 + b
    counts = np.bincount(key, minlength=NCORES * NG * 2).reshape(NCORES, NG, 2)
    cpg = np.maximum(1, _ceil_div(counts.max(axis=0), 128))  # [NG, 2]
    run_chunk_start = np.concatenate([[0], np.cumsum(cpg.reshape(-1))])
    tot_chunks = int(run_chunk_start[-1])
    tot_edges = tot_chunks * 128

    order = np.argsort(key, kind="stable")
    key_s = key[order]
    seg_start = np.searchsorted(key_s, np.arange(NCORES * NG * 2))
    rank = np.arange(src.size) - seg_start[key_s]
    gb = key_s % (NG * 2)
    pos = run_chunk_start[gb] * 128 + rank
    core_s = key_s // (NG * 2)

    cores = []
    for c in range(NCORES):
        m = core_s == c
        e = order[m]
        p = pos[m]
        esrc16 = np.zeros(tot_edges, np.int16)
        edstf = np.full(tot_edges, PADDST, np.float32)
        esrc16[p] = (src[e] - b[e] * HALF).astype(np.int16)
        edstf[p] = (dloc[e] % GRP).astype(np.float32)
        edstf = edstf.astype(BF16)
        cores.append(dict(
            esrc16=_wrap_idx16(esrc16),
            edstf=edstf.reshape(tot_chunks, 128).T.copy(),
            edstf_flat=edstf.reshape(1, tot_edges).copy(),
        ))
    return cpg, cores


def _build_head_schedule(x1, x2):
    """4 buckets by (x1>=HALF, x2>=HALF); uniform chunk counts across cores."""
    x1 = np.asarray(x1, np.int64)
    x2 = np.asarray(x2, np.int64)
    PC = P // NCORES
    q_all = (x1 >= HALF).astype(np.int64) + 2 * (x2 >= HALF).astype(np.int64)
    pb = np.ones(4, np.int64)
    for c in range(NCORES):
        cnt = np.bincount(q_all[c * PC : (c + 1) * PC], minlength=4)
        pb = np.maximum(pb, _ceil_div(cnt, 128))
    starts = np.concatenate([[0], np.cumsum(pb)]) * 128
    tot = int(starts[-1])
    percore = []
    for c in range(NCORES):
        sl = slice(c * PC, (c + 1) * PC)
        q = q_all[sl]
        x1c, x2c = x1[sl], x2[sl]
        x1_16 = np.zeros(tot, np.int16)
        x2_16 = np.zeros(tot, np.int16)
        posmap = np.zeros(PC, np.int64)
        for qq in range(4):
            m = q == qq
            n = int(m.sum())
            p = starts[qq] + np.arange(n)
            x1_16[p] = (x1c[m] - (qq & 1) * HALF).astype(np.int16)
            x2_16[p] = (x2c[m] - ((qq >> 1) & 1) * HALF).astype(np.int16)
            posmap[np.nonzero(m)[0]] = p
        percore.append(dict(
            x1_16=_wrap_idx16(x1_16),
            x2_16=_wrap_idx16(x2_16),
            posmap=posmap,
        ))
    return pb, percore


def _build_program(cpg, pb):
    import concourse.bacc as bacc
    import concourse.bass as bass
    import concourse.mybir as mybir
    import concourse.tile as tile

    F32 = mybir.dt.float32
    BF = mybir.dt.float16
    I16 = mybir.dt.int16
    AF = mybir.ActivationFunctionType
    OP = mybir.AluOpType

    NG = _ng()
    NKCH = _ceil_div(N, 128)        # node chunks for ftab builds
    NPAD = NKCH * 128
    tot_chunks = int(cpg.sum())
    tot_e16 = tot_chunks * 8        # idx cols (128 edges -> 8 cols of 16)
    tot_edges = tot_chunks * 128
    hb = int(pb.sum())              # head chunks per core
    TOTP = hb * 128
    KC = 8                          # ftab staging chunks per DMA

    nc = bacc.Bacc("TRN2", target_bir_lowering=False, debug=False,
                   num_devices=NCORES, num_swdge_queues=4)

    # ---- I/O ----
    hT_d = nc.dram_tensor("hT", [128, N], BF, kind="ExternalInput")
    hTs_d = nc.dram_tensor("hTs", [128, NS], BF, kind="ExternalInput")
    wcat_d = [nc.dram_tensor(f"wcat{l}", [128, RFB], BF, kind="ExternalInput")
              for l in range(2)]
    wer_d = [nc.dram_tensor(f"wer{l}", [128, 2], BF, kind="ExternalInput")
             for l in range(2)]
    w1_d = nc.dram_tensor("w1", [3 * NH, NH], BF, kind="ExternalInput")
    b1_d = nc.dram_tensor("b1", [NH, 1], F32, kind="ExternalInput")
    w2_d = nc.dram_tensor("w2", [NH, C], BF, kind="ExternalInput")
    b2_d = nc.dram_tensor("b2", [C, 1], F32, kind="ExternalInput")
    iota_d = nc.dram_tensor("iota", [128, RCAP * 128], BF, kind="ExternalInput")
    colv_d = nc.dram_tensor("colv", [128, RCAP * 128], BF, kind="ExternalInput")
    ident_d = nc.dram_tensor("ident", [128, 128], BF, kind="ExternalInput")
    esrc_d = nc.dram_tensor("esrc", [128, tot_e16], I16, kind="ExternalInput")
    edstf_d = nc.dram_tensor("edstf", [128, tot_chunks], BF,
                             kind="ExternalInput")
    edstfl_d = nc.dram_tensor("edstfl", [1, tot_edges], BF,
                              kind="ExternalInput")
    x1_d = nc.dram_tensor("x1i", [128, hb * 8], I16, kind="ExternalInput")
    x2_d = nc.dram_tensor("x2i", [128, hb * 8], I16, kind="ExternalInput")
    out_d = nc.dram_tensor("headout", [C, TOTP], F32, kind="ExternalOutput")

    with tile.TileContext(nc) as tc:
        with (
            tc.tile_pool(name="dram", bufs=1, space="DRAM") as dp,
            tc.tile_pool(name="cst", bufs=1) as cst,
            tc.tile_pool(name="ld", bufs=3) as ld,
            tc.tile_pool(name="st", bufs=2) as st,
            tc.tile_pool(name="gtp", bufs=4) as gtp,
            tc.tile_pool(name="ed", bufs=3) as ed,
            tc.tile_pool(name="hg", bufs=2) as hgp,
            tc.tile_pool(name="sc", bufs=6) as sc,
            tc.tile_pool(name="psA", bufs=3, space="PSUM") as psA,
            tc.tile_pool(name="psB", bufs=2, space="PSUM") as psB,
            tc.tile_pool(name="psE", bufs=3, space="PSUM") as psE,
        ):
            # ---- internal DRAM (pool tiles so Tile tracks RAW deps) ----
            ftab = [dp.tile([NPAD, RFB], BF, name=f"ftab{l}") for l in range(2)]
            hshard = [dp.tile([NS, 128], BF, name=f"hshard{l}")
                      for l in range(2)]
            hfull = dp.tile([N, 128], BF, name="hfull", addr_space="Shared")
            h2full = dp.tile([N, 128], BF, name="h2full", addr_space="Shared")

            # ---- constants ----
            iota_t = cst.tile([128, RCAP, 128], BF)
            nc.sync.dma_start(iota_t[:], iota_d[:].rearrange("p (c x) -> p c x", x=128))
            colv_t = cst.tile([128, RCAP, 128], BF)
            nc.sync.dma_start(colv_t[:], colv_d[:].rearrange("p (c x) -> p c x", x=128))
            ident_t = cst.tile([128, 128], BF)
            nc.sync.dma_start(ident_t[:], ident_d[:])
            wcat_t, wer_t = [], []
            for l in range(2):
                w = cst.tile([128, RFB], BF, name=f"wcat{l}t")
                nc.sync.dma_start(w[:], wcat_d[l][:])
                wcat_t.append(w)
                w = cst.tile([128, 2], BF, name=f"wer{l}t")
                nc.sync.dma_start(w[:], wer_d[l][:])
                wer_t.append(w)
            er_all = [cst.tile([128, NG, 2], BF, name=f"erall{l}")
                      for l in range(2)]
            w1_t = cst.tile([128, 3, NH], BF)
            for j in range(3):
                nc.sync.dma_start(w1_t[:, j, :], w1_d[j * 128 : (j + 1) * 128, :])
            b1_t = cst.tile([NH, 1], F32)
            nc.sync.dma_start(b1_t[:], b1_d[:])
            w2_t = cst.tile([NH, C], BF)
            nc.sync.dma_start(w2_t[:], w2_d[:])
            b2_t = cst.tile([C, 1], F32)
            nc.sync.dma_start(b2_t[:], b2_d[:])
            edstf_t = cst.tile([128, tot_chunks], BF)
            nc.sync.dma_start(edstf_t[:], edstf_d[:])
            esrc_t = cst.tile([128, tot_e16], I16)
            nc.sync.dma_start(esrc_t[:], esrc_d[:])

            qn = [0]  # SWDGE queue round-robin cursor

            def build_ftab(l):
                """ftab[l] = h_l @ wcat[l]; layer 0 reads hT (pre-transposed),
                layer 1 PE-transposes hfull row chunks."""
                for k0 in range(0, NKCH, KC):
                    kn = min(KC, NKCH - k0)
                    stg = st.tile([128, KC, RFB], BF, tag="ftstg")
                    for kk in range(kn):
                        k = k0 + kk
                        rows = min(128, N - k * 128)
                        lhs = ld.tile([128, 128], BF, tag="lhs")
                        if l == 0:
                            if rows < 128:
                                nc.vector.memset(lhs[:], 0.0)
                            nc.sync.dma_start(lhs[:, :rows],
                                              hT_d[:, k * 128 : k * 128 + rows])
                        else:
                            hrow = ld.tile([128, 128], BF, tag="hrow")
                            if rows < 128:
                                nc.vector.memset(hrow[:], 0.0)
                            nc.sync.dma_start(
                                hrow[:rows, :],
                                hfull[k * 128 : k * 128 + rows, :])
                            ptr = psB.tile([128, 128], BF, tag="tr")
                            nc.tensor.transpose(ptr[:], hrow[:], ident_t[:])
                            nc.vector.tensor_copy(lhs[:], ptr[:])
                        pchunk = psA.tile([128, RFB], F32, tag="acc")
                        nc.tensor.matmul(pchunk[:], lhs[:], wcat_t[l][:],
                                         start=True, stop=True)
                        nc.scalar.activation(stg[:, kk, :], pchunk[:], AF.Copy)
                    dst = ftab[l][k0 * 128 : (k0 + kn) * 128, :]
                    nc.sync.dma_start(
                        dst.rearrange("(k p) r -> p k r", p=128), stg[:, :kn, :])

            def build_er(l):
                """er_all[l][:, g, :] = er for the core's dst shard."""
                for k in range(NG):
                    rows = min(128, NS - k * 128)
                    lhs = ld.tile([128, 128], BF, tag="lhs")
                    if l == 0:
                        if rows < 128:
                            nc.vector.memset(lhs[:], 0.0)
                        nc.sync.dma_start(lhs[:, :rows],
                                          hTs_d[:, k * 128 : k * 128 + rows])
                    else:
                        hrow = ld.tile([128, 128], BF, tag="hrow")
                        if rows < 128:
                            nc.vector.memset(hrow[:], 0.0)
                        nc.sync.dma_start(
                            hrow[:rows, :],
                            hshard[0][k * 128 : k * 128 + rows, :])
                        ptr = psB.tile([128, 128], BF, tag="tr")
                        nc.tensor.transpose(ptr[:], hrow[:], ident_t[:])
                        nc.vector.tensor_copy(lhs[:], ptr[:])
                    perp = psE.tile([128, 2], F32, tag="erp")
                    nc.tensor.matmul(perp[:], lhs[:], wer_t[l][:],
                                     start=True, stop=True)
                    nc.vector.tensor_copy(er_all[l][:, k, :], perp[:])

            def edge_phase(l):
                """One GAT aggregation layer; writes hshard[l]."""
                cc = 0  # global chunk cursor
                for g in range(NG):
                    gpsum = psA.tile([128, NAGG], F32, tag="acc")
                    nchunks_g = int(cpg[g].sum())
                    done = 0
                    for b in range(2):
                        base = b * HALF
                        nrows = (N - HALF) if b else HALF
                        nrem = int(cpg[g, b])
                        while nrem > 0:
                            n = min(RCAP, nrem)
                            nrem -= n
                            ne = n * 128
                            gt = gtp.tile([128, RCAP, RFB], BF, tag="gt")
                            nc.gpsimd.dma_gather(
                                gt[:, :n, :], ftab[l][base : base + nrows, :],
                                esrc_t[:, cc * 8 : (cc + n) * 8], ne, ne, RFB,
                                queue_num=qn[0] % 4)
                            qn[0] += 1
                            # replicate dst ids across partitions for ohT
                            eb = ed.tile([128, RCAP, 128], BF, tag="eb")
                            nc.sync.dma_start(
                                eb[:, :n, :],
                                bass.AP(edstfl_d, cc * 128, [[0, 128], [1, ne]]))
                            # oh[e, c, d] = (iota[d] == dstf[e, c])
                            oh = ed.tile([128, RCAP, 128], BF, tag="oh")
                            e0 = edstf_t[:]
                            edstf_b = bass.AP(e0.tensor, e0.offset + cc,
                                              [e0.ap[0], [1, n], [0, 128]])
                            nc.vector.tensor_tensor(oh[:, :n, :],
                                                    iota_t[:, :n, :],
                                                    edstf_b, op=OP.is_equal)
                            # ohT[d, c, e] = (colv[d] == dstfB[d, c, e])
                            ohT = ed.tile([128, RCAP, 128], BF, tag="ohT")
                            nc.vector.tensor_tensor(ohT[:, :n, :],
                                                    colv_t[:, :n, :],
                                                    eb[:, :n, :],
                                                    op=OP.is_equal)
                            # er per edge: erps[:, 2c:2c+2] = ohT_c^T @ er_g
                            erps = psE.tile([128, 2 * RCAP], F32, tag="erp")
                            for i in range(n):
                                nc.tensor.matmul(erps[:, 2 * i : 2 * i + 2],
                                                 ohT[:, i, :],
                                                 er_all[l][:, g, :],
                                                 start=True, stop=True)
                            # s = exp(lrelu(el + er)); el at gt cols 256:258
                            s_t = sc.tile([128, RCAP, 2], F32, tag="s")
                            erv = erps[:]
                            er_ap = bass.AP(erv.tensor, erv.offset,
                                            [erv.ap[0], [2, n], [1, 2]])
                            nc.vector.tensor_tensor(
                                s_t[:, :n, :], gt[:, :n, 256:258], er_ap,
                                op=OP.add)
                            lr = sc.tile([128, RCAP, 2], F32, tag="lr")
                            nc.scalar.activation(lr[:, :n, :], s_t[:, :n, :],
                                                 AF.Exp)
                            nc.scalar.activation(s_t[:, :n, :], s_t[:, :n, :],
                                                 AF.Exp, scale=NEG)
                            nc.vector.tensor_tensor(gt[:, :n, 258:260],
                                                    lr[:, :n, :], s_t[:, :n, :],
                                                    op=OP.max)
                            # msg = feat * s (broadcast over each head's cols)
                            g0 = gt[:]
                            feat_ap = bass.AP(
                                g0.tensor, g0.offset,
                                [g0.ap[0], [RFB, n], [128, 2], [1, 128]])
                            s_ap = bass.AP(
                                g0.tensor, g0.offset + 258,
                                [g0.ap[0], [RFB, n], [1, 2], [0, 128]])
                            nc.vector.tensor_tensor(feat_ap, feat_ap, s_ap,
                                                    op=OP.mult)
                            for i in range(n):
                                nc.tensor.matmul(gpsum[:], oh[:, i, :],
                                                 gt[:, i, 0:NAGG],
                                                 start=(done == 0),
                                                 stop=(done == nchunks_g - 1))
                                done += 1
                                cc += 1
                    # postprocess group -> h rows
                    den = sc.tile([128, 2], F32, tag="den")
                    nc.vector.tensor_scalar_add(den[:], gpsum[:, 258:260], 1e-9)
                    rec = sc.tile([128, 2], F32, tag="rec")
                    nc.vector.reciprocal(rec[:], den[:])
                    r0 = sc.tile([128, 128], F32, tag="r0")
                    nc.scalar.activation(r0[:], gpsum[:, 0:128], AF.Relu,
                                         scale=rec[:, 0:1])
                    r1 = sc.tile([128, 128], F32, tag="r1")
                    nc.scalar.activation(r1[:], gpsum[:, 128:256], AF.Relu,
                                         scale=rec[:, 1:2])
                    hsum = sc.tile([128, 128], F32, tag="hsum")
                    nc.vector.tensor_tensor(hsum[:], r0[:], r1[:], op=OP.add)
                    hrow = sc.tile([128, 128], BF, tag="hmean")
                    nc.scalar.activation(hrow[:], hsum[:], AF.Copy, scale=0.5)
                    rows = min(128, NS - g * 128)
                    nc.sync.dma_start(hshard[l][g * 128 : g * 128 + rows, :],
                                      hrow[:rows, :])

            def allgather(l):
                dst = hfull if l == 0 else h2full
                nc.gpsimd.collective_compute(
                    "AllGather", mybir.AluOpType.bypass,
                    ins=[hshard[l][:, :].opt()], outs=[dst[:, :].opt()],
                    replica_groups=[list(range(NCORES))])

            def head_phase():
                x1_t = cst.tile([128, hb * 8], I16, tag="x1i")
                nc.sync.dma_start(x1_t[:], x1_d[:])
                x2_t = cst.tile([128, hb * 8], I16, tag="x2i")
                nc.sync.dma_start(x2_t[:], x2_d[:])
                outsb = cst.tile([C, TOTP], F32, tag="outsb")
                hc = 0
                for q in range(4):
                    base1 = (q & 1) * HALF
                    base2 = ((q >> 1) & 1) * HALF
                    nr1 = (N - HALF) if (q & 1) else HALF
                    nr2 = (N - HALF) if ((q >> 1) & 1) else HALF
                    nrem = int(pb[q])
                    while nrem > 0:
                        n = min(RCAP, nrem)
                        nrem -= n
                        ne = n * 128
                        g1 = hgp.tile([128, RCAP, 128], BF, tag="hg1")
                        g2 = hgp.tile([128, RCAP, 128], BF, tag="hg2")
                        nc.gpsimd.dma_gather(
                            g1[:, :n, :], h2full[base1 : base1 + nr1, :],
                            x1_t[:, hc * 8 : (hc + n) * 8], ne, ne, 128,
                            queue_num=qn[0] % 4)
                        qn[0] += 1
                        nc.gpsimd.dma_gather(
                            g2[:, :n, :], h2full[base2 : base2 + nr2, :],
                            x2_t[:, hc * 8 : (hc + n) * 8], ne, ne, 128,
                            queue_num=qn[0] % 4)
                        qn[0] += 1
                        dt_ = hgp.tile([128, RCAP, 128], BF, tag="hd")
                        nc.vector.tensor_tensor(dt_[:, :n, :], g1[:, :n, :],
                                                g2[:, :n, :], op=OP.subtract)
                        nc.scalar.activation(dt_[:, :n, :], dt_[:, :n, :],
                                             AF.Abs)
                        for i in range(n):
                            po1 = psA.tile([128, 128], F32, tag="acc")
                            for j, tsrc in enumerate((g1, g2, dt_)):
                                ptr = psB.tile([128, 128], BF, tag="tr")
                                nc.tensor.transpose(ptr[:], tsrc[:, i, :],
                                                    ident_t[:])
                                tsb = sc.tile([128, 128], BF, tag="htsb")
                                nc.vector.tensor_copy(tsb[:], ptr[:])
                                nc.tensor.matmul(po1[:], w1_t[:, j, :], tsb[:],
                                                 start=(j == 0), stop=(j == 2))
                            o1 = sc.tile([128, 128], BF, tag="ho1")
                            nc.scalar.activation(o1[:], po1[:], AF.Relu,
                                                 bias=b1_t[:, 0:1])
                            po2 = psA.tile([C, 128], F32, tag="acc")
                            nc.tensor.matmul(po2[:], w2_t[:], o1[:],
                                             start=True, stop=True)
                            nc.vector.tensor_scalar(
                                outsb[:, hc * 128 : (hc + 1) * 128], po2[:],
                                b2_t[:, 0:1], None, OP.add)
                            hc += 1
                nc.sync.dma_start(out_d[:], outsb[:])

            build_ftab(0)
            build_er(0)
            edge_phase(0)
            allgather(0)
            build_ftab(1)
            build_er(1)
            edge_phase(1)
            allgather(1)
            head_phase()

    nc.compile()
    return nc


def _prepare_inputs(src, dst, h, x1, x2, W0, al0, ar0, W1, al1, ar1,
                    w1, b1, w2, b2):
    cpg, ecores = _build_edge_schedule(src, dst)
    pb, hcores = _build_head_schedule(x1, x2)

    Wcat0, Wer0 = _fold_weights(np.asarray(W0, np.float32),
                                np.asarray(al0, np.float32),
                                np.asarray(ar0, np.float32))
    Wcat1, Wer1 = _fold_weights(np.asarray(W1, np.float32),
                                np.asarray(al1, np.float32),
                                np.asarray(ar1, np.float32))
    hT = np.ascontiguousarray(np.asarray(h, np.float32).T).astype(BF16)
    iota = np.tile(np.arange(128, dtype=np.float32), (128, RCAP)).astype(BF16)
    colv = np.tile(np.arange(128, dtype=np.float32)[:, None],
                   (1, RCAP * 128)).astype(BF16)
    ident = np.eye(128, dtype=np.float32).astype(BF16)

    in_maps = []
    for c in range(NCORES):
        ec, hcj = ecores[c], hcores[c]
        in_maps.append({
            "hT": hT,
            "hTs": np.ascontiguousarray(hT[:, c * NS : (c + 1) * NS]),
            "wcat0": Wcat0, "wer0": Wer0, "wcat1": Wcat1, "wer1": Wer1,
            "w1": np.asarray(w1, np.float32).astype(BF16),
            "b1": np.asarray(b1, np.float32).reshape(NH, 1),
            "w2": np.asarray(w2, np.float32).astype(BF16),
            "b2": np.asarray(b2, np.float32).reshape(C, 1),
            "iota": iota, "colv": colv, "ident": ident,
            "esrc": ec["esrc16"], "edstf": ec["edstf"],
            "edstfl": ec["edstf_flat"],
            "x1i": hcj["x1_16"], "x2i": hcj["x2_16"],
        })
    return cpg, pb, in_maps, hcores


def kernel(src, dst, h, x1, x2, W0, al0, ar0, W1, al1, ar1, w1, b1, w2, b2):
    src = np.asarray(src, np.int64)
    dst = np.asarray(dst, np.int64)
    x1 = np.asarray(x1, np.int64)
    x2 = np.asarray(x2, np.int64)

    cpg, pb, in_maps, hcores = _prepare_inputs(
        src, dst, h, x1, x2, W0, al0, ar0, W1, al1, ar1, w1, b1, w2, b2)

    key = (cpg.tobytes(), pb.tobytes())
    if key not in _CACHE:
        _CACHE.clear()
        _CACHE[key] = _build_program(cpg, pb)
    nc = _CACHE[key]

    from concourse.bass_utils import run_bass_kernel_spmd
    kw = {"trace": True} if _TRACE else {}
    res = run_bass_kernel_spmd(nc, in_maps, core_ids=list(range(NCORES)), **kw)
    global LAST_PERF
    LAST_PERF = res

    PC = P // NCORES
    out = np.empty((P, C), np.float32)
    for c in range(NCORES):
        cols = res.results[c]["headout"]          # [C, TOTP]
        out[c * PC : (c + 1) * PC, :] = cols[:, hcores[c]["posmap"]].T
    return out
